# revision 1
# baseline (speedup 1.0000x reference)
"""CondConv_Spatial Trainium2 kernel.

Data-parallel across batch: 8 samples -> 8 NeuronCores, one sample per core.

Per-core algorithm (x: [64, 256, 256] f32):
  1. x resident in SBUF as [128 part, 129 rows, 258 cols]:
       partitions (c, half): half 0 = global rows 0..128, half 1 = rows 127..255
       width padded 256->258 with zero cols 0 and 257 (free conv borders)
  2. attention: row sums (POOL reduce) + column sums (DVE add-tree),
     cross-partition sums via tiny ones-matmuls, sigmoid(net_w @ sums / 16384)
  3. agg_w = sum_k att[k] * W_k (DVE), agg_b likewise
  4. conv 3x3 as fp32r matmuls accumulating in PSUM, bank tile = 2 rows:
       psum slot j: 3 M-packed pair matmuls (lhsT [64,128] = [W(-1,dx)|W(+1,dx)],
         rhs = input row j-1+half): psumA[0:64] += dy=-1 part for out row j,
         psumB[64:128] += dy=+1 part for out row j-2;
       + 3 singles (W(0,dx), rhs = input row j+half) into psumA[0:64].
       Interior tiles use N=512 matmuls spanning both bank slots (2D rhs AP).
     fold (bank-aligned, 2 rows/op): out = psumA_tile[t] + agg_b (ACT Identity
       +bias) + psumB_tile[t+1] (DVE add); 4-row staged output DMAs.
     top half on PE row-strips (0,*), bottom on (64,*) -> concurrent matmuls
"""

import sys

sys.path.insert(0, "/opt/trn_rl_repo")

from contextlib import ExitStack

import numpy as np

import concourse.bass as bass
import concourse.tile as tile
from concourse import bacc, bass_utils, mybir

F32 = mybir.dt.float32
F32R = mybir.dt.float32r
AF = mybir.ActivationFunctionType
ALU = mybir.AluOpType

BS, C, H, W = 8, 64, 256, 256
K, O, I, KS = 4, 64, 64, 3
HH = 129  # rows per half (halves overlap at global rows 127/128)
WP = 258  # padded width
NCORES = 8


def _emit_body(ctx: ExitStack, tc: "tile.TileContext", x_d, netw_d, wgt_d, bias_d, out_d, attsc_d, att4_d):
    nc = tc.nc

    const_pool = ctx.enter_context(tc.tile_pool(name="const", bufs=1))
    xpool = ctx.enter_context(tc.tile_pool(name="xres", bufs=1))
    redpool = ctx.enter_context(tc.tile_pool(name="red", bufs=1))

    # ---------------- resident x + weight/bias loads ----------------
    x_sb = xpool.tile([128, HH, WP], F32R)
    x_f = x_sb.bitcast(F32)  # f32 view for DVE/ACT attention ops
    nc.gpsimd.memset(x_f[:, :, 0:1], 0.0)
    nc.gpsimd.memset(x_f[:, :, 257:258], 0.0)

    # W_all[p=(dup,i), k, t, o]; tap order t:
    # (-1,-1),(0,-1),(-1,0),(0,0),(-1,1),(0,1),(1,-1),(1,0),(1,1)
    # tap order: (-1,-1),(0,-1),(-1,0),(0,0),(-1,1),(0,1),(1,-1),(1,0),(1,1)
    TAPS = [(-1, -1), (1, -1), (-1, 0), (1, 0), (-1, 1), (1, 1), (0, -1), (0, 0), (0, 1)]
    w_all = const_pool.tile([128, K, 9, O], F32)
    w_io = wgt_d.rearrange("k o i ky kx -> k ky kx i o")  # [K, 3, 3, 64, 64]
    for k in range(K):
        for d in range(2):
            for t, (dy, dx) in enumerate(TAPS):
                nc.sync.dma_start(
                    w_all[64 * d : 64 * d + 64, k, t, :], w_io[k, dy + 1, dx + 1]
                )

    bias_sb = const_pool.tile([128, K], F32)
    bias_t = bias_d.rearrange("k o -> o k")
    for d in range(2):
        nc.sync.dma_start(bias_sb[64 * d : 64 * d + 64, :], bias_t)

    # net_w chunks for the K=128 contraction: [p=j1, j2, k]
    netw_sb = const_pool.tile([128, 4, K], F32)
    netw_r = netw_d.rearrange("k (j2 j1) -> j1 j2 k", j1=128)
    for j2 in range(4):
        nc.sync.dma_start(netw_sb[:, j2, :], netw_r[:, j2, :])

    ones_sb = const_pool.tile([128, 1], F32)
    nc.gpsimd.memset(ones_sb[:, :], 1.0)

    # attention scratch
    rowsum = redpool.tile([128, HH], F32)
    colsum = redpool.tile([128, 256], F32)
    s8 = redpool.tile([128, 8, 256], F32)
    nc.gpsimd.memset(colsum[:, :], 0.0)

    # chunked load: 8 chunks of 16 rows + row 128
    chunks = [(a, min(a + 16, HH)) for a in range(0, HH, 16)]  # last = (128, 129)
    for (a, b) in chunks:
        for h in range(2):
            g0 = 0 if h == 0 else 127
            nc.sync.dma_start(
                x_sb[64 * h : 64 * h + 64, a:b, 1:257], x_d[:, g0 + a : g0 + b, :]
            )
        # row sums for this chunk: single DVE reduce over the inner 256 cols
        nc.vector.reduce_sum(rowsum[:, a:b], x_f[:, a:b, 1:257], axis=mybir.AxisListType.X)
        # column sums: in-chunk add tree on DVE (pad cols excluded)
        if b - a == 16:
            nc.gpsimd.tensor_tensor(s8[:, :, :], x_f[:, a : a + 8, 1:257], x_f[:, a + 8 : a + 16, 1:257], op=ALU.add)
            nc.vector.tensor_tensor(s8[:, 0:4, :], s8[:, 0:4, :], s8[:, 4:8, :], op=ALU.add)
            nc.vector.tensor_tensor(s8[:, 0:2, :], s8[:, 0:2, :], s8[:, 2:4, :], op=ALU.add)
            nc.vector.tensor_tensor(s8[:, 0:1, :], s8[:, 0:1, :], s8[:, 1:2, :], op=ALU.add)
            nc.vector.tensor_tensor(colsum[:, :], colsum[:, :], s8[:, 0, :], op=ALU.add)
        else:
            for r in range(a, b):
                nc.vector.tensor_tensor(colsum[:, :], colsum[:, :], x_f[:, r, 1:257], op=ALU.add)

    # halves overlap at global rows 127,128 -> remove one copy of each
    nc.vector.tensor_tensor(colsum[0:64, :], colsum[0:64, :], x_f[0:64, 128, 1:257], op=ALU.subtract)
    nc.vector.tensor_tensor(colsum[64:128, :], colsum[64:128, :], x_f[64:128, 0, 1:257], op=ALU.subtract)

    # ---------------- attention -> att[4] ----------------
    with tc.tile_pool(name="attps", bufs=1, space="PSUM") as attps:
        att_sb = redpool.tile([1, 512], F32)
        ps_h = attps.tile([1, 256], F32, tag="attps", bufs=2)
        nc.tensor.matmul(ps_h[:, 0:128], ones_sb[0:64, :], rowsum[0:64, 0:128], start=True, stop=True)
        nc.tensor.matmul(ps_h[:, 128:256], ones_sb[64:128, :], rowsum[64:128, 1:129], start=True, stop=True)
        ps_v = attps.tile([1, 256], F32, tag="attps", bufs=2)
        nc.tensor.matmul(ps_v[:, :], ones_sb[:, :], colsum[:, 0:256], start=True, stop=True)
        nc.scalar.copy(att_sb[:, 0:256], ps_h[:, :])
        nc.scalar.copy(att_sb[:, 256:512], ps_v[:, :])

        # bounce through DRAM to repartition [1,512] -> [128,4]
        nc.sync.dma_start(attsc_d[0:1, :], att_sb[0:1, :])
        att_t = redpool.tile([128, 4], F32)
        nc.sync.dma_start(att_t[:, :], attsc_d[0].rearrange("(j2 j1) -> j1 j2", j1=128))

        ps_att = attps.tile([4, 1], F32, tag="attps", bufs=2)
        for j2 in range(4):
            nc.tensor.matmul(ps_att[:, :], netw_sb[:, j2, :], att_t[:, j2 : j2 + 1], start=(j2 == 0), stop=(j2 == 3))
        att4_sb = redpool.tile([4, 1], F32)
        nc.scalar.activation(att4_sb[:, :], ps_att[:, :], AF.Sigmoid, scale=1.0 / 16384.0)

        # broadcast att to all partitions: bounce to a [1,4] row, then K=1 ones-matmul
        nc.sync.dma_start(att4_d[:, :], att4_sb[:, :])
        att_row = redpool.tile([1, K], F32)
        nc.sync.dma_start(att_row[0:1, :], att4_d.rearrange("a b -> b a"))
        ones1 = const_pool.tile([1, 128], F32)
        nc.gpsimd.memset(ones1[:, :], 1.0)
        ps_bc = attps.tile([128, K], F32, tag="attps", bufs=2)
        nc.tensor.matmul(ps_bc[:, :], ones1[:, :], att_row[:, :], start=True, stop=True)
        att_bc = redpool.tile([128, K], F32)
        nc.vector.tensor_copy(att_bc[:, :], ps_bc[:, :])

        # ---------------- aggregate weights / bias ----------------
        agg_w = const_pool.tile([128, 9, O], F32)
        nc.vector.tensor_scalar_mul(agg_w[:, :, :], w_all[:, 0, :, :], att_bc[:, 0:1])
        for k in range(1, K):
            nc.vector.scalar_tensor_tensor(
                agg_w[:, :, :], w_all[:, k, :, :], att_bc[:, k : k + 1], agg_w[:, :, :],
                op0=ALU.mult, op1=ALU.add,
            )
        tmp4 = redpool.tile([128, K], F32)
        nc.vector.tensor_tensor(tmp4[:, :], bias_sb[:, :], att_bc[:, :], op=ALU.mult)
        agg_b = const_pool.tile([128, 1], F32)
        nc.vector.reduce_sum(agg_b[:, :], tmp4[:, :], axis=mybir.AxisListType.X)
        agg_r = const_pool.tile([128, 9, O], F32R)
        nc.scalar.copy(agg_r[:, :, :], agg_w[:, :, :])

    # ---------------- conv ----------------
    # v2 scheme: psum slot j gets
    #   pairs (lhsT [64, 2, 64] = [W(-1,dx) | W(+1,dx)], rhs = input row j-1+beta):
    #     psumA_j[0:64]   += dy=-1 contribution for out row j
    #     psumB_j[64:128] += dy=+1 contribution for out row j-2
    #   singles (W(0,dx), rhs = input row j+beta) into psumA_j[0:64]
    # fold: out_r = psumA_r + psumB_{r+2} + agg_b  -> bank-aligned 2-row folds
    ptop = ctx.enter_context(tc.tile_pool(name="ptop", bufs=4, space="PSUM"))
    pbot = ctx.enter_context(tc.tile_pool(name="pbot", bufs=4, space="PSUM"))
    stpool = ctx.enter_context(tc.tile_pool(name="stage", bufs=4))

    pools = (ptop, pbot)
    stg4 = [None, None]
    NT = 65  # psum tiles per half; tile t = slots {2t, 2t+1}; out rows tiles 0..63
    tiles = [[None] * NT, [None] * NT]

    for t in range(NT):
        # ---- matmuls for tile t (slots 2t, 2t+1), both halves interleaved ----
        mms = [[], []]  # entries: (out_ap, lhsT, rhs, slots_covered)
        for h in range(2):
            beta = h
            pt = pools[h].tile([128, 2, 256], F32, name=f"ps{h}", tag=f"ps{h}")
            tiles[h][t] = pt
            pair_ok = [0 <= (j - 1 + beta) <= 128 for j in (2 * t, 2 * t + 1)]
            sing_ok = [j <= 127 for j in (2 * t, 2 * t + 1)]
            if all(pair_ok) and all(sing_ok):
                # uniform interior tile: N=512 matmuls spanning both slots
                L = 2 * t - 1 + beta
                for dxi in range(3):
                    mms[h].append((pt[:, :, :],
                                   agg_r[64 * h : 64 * h + 64, 2 * dxi : 2 * dxi + 2, :],
                                   x_sb[64 * h : 64 * h + 64, L : L + 2, dxi : dxi + 256],
                                   (0, 1)))
                L0 = 2 * t + beta
                for dxi in range(3):
                    mms[h].append((pt[0:64, :, :],
                                   agg_r[64 * h : 64 * h + 64, 6 + dxi, :],
                                   x_sb[64 * h : 64 * h + 64, L0 : L0 + 2, dxi : dxi + 256],
                                   (0, 1)))
            else:
                # edge tile: per-slot N=256 matmuls (pairs first per slot)
                for si, j in enumerate((2 * t, 2 * t + 1)):
                    L = j - 1 + beta
                    if pair_ok[si]:
                        for dxi in range(3):
                            mms[h].append((pt[:, si, :],
                                           agg_r[64 * h : 64 * h + 64, 2 * dxi : 2 * dxi + 2, :],
                                           x_sb[64 * h : 64 * h + 64, L, dxi : dxi + 256],
                                           (si,)))
                    if sing_ok[si]:
                        L0 = j + beta
                        for dxi in range(3):
                            mms[h].append((pt[0:64, si, :],
                                           agg_r[64 * h : 64 * h + 64, 6 + dxi, :],
                                           x_sb[64 * h : 64 * h + 64, L0, dxi : dxi + 256],
                                           (si,)))
        # start=True on the first matmul touching each slot; stop=True on the last
        flags = [[], []]
        for h in range(2):
            seen = set()
            last_for_slot = {}
            for i, (_, _, _, slots) in enumerate(mms[h]):
                for s in slots:
                    last_for_slot[s] = i
            for i, (_, _, _, slots) in enumerate(mms[h]):
                st = any(s not in seen for s in slots)
                seen.update(slots)
                sp = any(last_for_slot[s] == i for s in slots)
                flags[h].append((st, sp))
        # emit interleaved across halves for PE row-strip overlap
        nmax = max(len(mms[0]), len(mms[1]))
        for i in range(nmax):
            for h in range(2):
                if i < len(mms[h]):
                    o_ap, lhsT, rhs, _ = mms[h][i]
                    st, sp = flags[h][i]
                    nc.tensor.matmul(o_ap, lhsT, rhs, start=st, stop=sp, skip_group_check=True)

        # ---- fold + store for out-row tile t-1 (rows 2t-2, 2t-1) ----
        if t >= 1:
            rt = t - 1
            for h in range(2):
                if rt % 2 == 0:
                    stg4[h] = stpool.tile([64, 4, 256], F32, name=f"st{h}", tag=f"st{h}")
                stg = stg4[h][:, 2 * (rt % 2) : 2 * (rt % 2) + 2, :]
                nc.scalar.activation(stg[:, :, :], tiles[h][rt][0:64, :, :], AF.Identity, bias=agg_b[0:64, :])
                if h == 1 and rt == 63:
                    # bottom out row 127: dy=+1 is zero padding; only row 126 gets B
                    nc.vector.tensor_tensor(stg[:, 0, :], stg[:, 0, :], tiles[h][t][64:128, 0, :], op=ALU.add)
                else:
                    nc.vector.tensor_tensor(stg[:, :, :], stg[:, :, :], tiles[h][t][64:128, :, :], op=ALU.add)
                if rt % 2 == 1:
                    g = 2 * (rt - 1) + 128 * h
                    nc.sync.dma_start(out_d[:, g : g + 4, :], stg4[h][:, :, :])
                tiles[h][rt] = None  # release reference


def build_nc():
    nc = bacc.Bacc("TRN2", target_bir_lowering=False, debug=False)
    x_d = nc.dram_tensor("x", [C, H, W], F32R, kind="ExternalInput").ap()
    netw_d = nc.dram_tensor("net_w", [K, H + W], F32, kind="ExternalInput").ap()
    wgt_d = nc.dram_tensor("weight", [K, O, I, KS, KS], F32, kind="ExternalInput").ap()
    bias_d = nc.dram_tensor("bias", [K, O], F32, kind="ExternalInput").ap()
    out_d = nc.dram_tensor("out", [O, H, W], F32, kind="ExternalOutput").ap()
    attsc_d = nc.dram_tensor("attsc", [1, 512], F32).ap()
    att4_d = nc.dram_tensor("att4", [K, 1], F32).ap()

    with tile.TileContext(nc) as tc, ExitStack() as ctx:
        _emit_body(ctx, tc, x_d, netw_d, wgt_d, bias_d, out_d, attsc_d, att4_d)
    nc.compile()
    return nc


_NC_CACHE = {}


def get_nc():
    if "nc" not in _NC_CACHE:
        _NC_CACHE["nc"] = build_nc()
    return _NC_CACHE["nc"]


def run(x, net_w, weight, bias, trace=False, trace_kwargs=None):
    nc = get_nc()
    net_w = np.ascontiguousarray(net_w, dtype=np.float32)
    weight = np.ascontiguousarray(weight, dtype=np.float32)
    bias = np.ascontiguousarray(bias, dtype=np.float32)
    in_maps = [
        {
            "x": np.ascontiguousarray(x[b], dtype=np.float32),
            "net_w": net_w,
            "weight": weight,
            "bias": bias,
        }
        for b in range(BS)
    ]
    res = bass_utils.run_bass_kernel_spmd(
        nc, in_maps, core_ids=list(range(NCORES)), trace=trace,
        **(trace_kwargs or {}),
    )
    out = np.stack([res.results[b]["out"] for b in range(BS)]).astype(np.float32)
    return out, res


def kernel(x, net_w, weight, bias):
    out, _ = run(x, net_w, weight, bias)
    return out

